# revision 79
# baseline (speedup 1.0000x reference)
"""Trainium2 Bass kernel for causal self-attention (GPT-J RoPE), 8-way
tensor-parallel over heads.

Contract: kernel(x, W_qkv, W_proj) -> np.ndarray  (full [T, D] output)

Sharding: 16 heads / 8 cores = 2 heads per core. Each core computes its
2 heads' QKV projection, RoPE, causal attention, and its partial
W_proj contribution; the host sums the 8 bf16 partial outputs in fp32
(the TP all-reduce), which is the unshard step.

Per-core device program (all matmul inputs bf16, fp32 PSUM accum):
  Software-pipelined over 512-query blocks: iteration i emits, woven
  together per-engine, (a) attention+proj for block i-1, (b) score
  matmuls + exp for block i, (c) QKV+RoPE+V-transpose for block i+1.
  - scores: per (i, key-tile pair) [tk=128, w] PSUM, exp on ScalarE
    (scale=1/sqrt(C), no max subtraction), diagonal masking via gpsimd
    affine_select on the bf16 est tiles.
  - AV: out [tq=128, 65] per (query-tile, key-tile) with a ones column
    in V providing the softmax denominator; normalize is fused into the
    PSUM->SBUF evict via tensor_scalar(mul, reciprocal).
  - proj: both heads' O^T stacked to [c2=128, tq] via one PE transpose,
    single K=128 matmul per 512-wide output chunk.
  PSUM (8 banks): score pairs [128,1024]x2 = 4; qkv/rope/vT share a
  rotating [128,512]x2 = 2; AV/transpose/proj share a rotating x2 = 2.
"""

import math
import sys

if "/opt/trn_rl_repo" not in sys.path:
    sys.path.insert(0, "/opt/trn_rl_repo")

import numpy as np
import ml_dtypes

import concourse.bass as bass  # noqa: F401
import concourse.mybir as mybir
import concourse.tile as tile
from concourse import bacc
from concourse.bass_utils import run_bass_kernel_spmd
from concourse.masks import make_identity

F32 = mybir.dt.float32
BF16 = mybir.dt.bfloat16

N_CORES = 8
N_HEAD = 16
T_FULL = 4096
D_FULL = 1024
C_HEAD = 64


def build_program(T=4096, D=1024, C=64, num_devices=8):
    TQB = 512
    C2 = 2 * C                # 128: both heads' channels
    ND = D // 128             # d-tiles for the QKV contraction (8)
    NI = T // TQB             # 512-query blocks (8)
    NT = T // 128             # 128-wide key tiles (32)
    scale = 1.0 / math.sqrt(C)

    nc = bacc.Bacc(
        "TRN2",
        target_bir_lowering=False,
        debug=False,
        enable_asserts=False,
        num_devices=num_devices,
    )

    xT_d = nc.dram_tensor("xT", [D, T], BF16, kind="ExternalInput").ap()
    # weights packed [128, D]: col block d holds the d-th 128-row slice
    wq_d = nc.dram_tensor("wq", [C2, D], BF16, kind="ExternalInput").ap()
    wk_d = nc.dram_tensor("wk", [C2, D], BF16, kind="ExternalInput").ap()
    wv_d = nc.dram_tensor("wv", [C2, D], BF16, kind="ExternalInput").ap()
    perm_d = nc.dram_tensor("perm", [C2, C2], BF16, kind="ExternalInput").ap()
    cos_d = nc.dram_tensor("cosT", [C2, T], BF16, kind="ExternalInput").ap()
    sin_d = nc.dram_tensor("sinT", [C2, T], BF16, kind="ExternalInput").ap()
    wp_d = nc.dram_tensor("wp", [C2, D], BF16, kind="ExternalInput").ap()
    out_d = nc.dram_tensor("out", [T, D], BF16, kind="ExternalOutput").ap()

    with tile.TileContext(nc) as tc:
        with (
            tc.tile_pool(name="const", bufs=1) as pconst,
            tc.tile_pool(name="xs", bufs=18) as px,
            tc.tile_pool(name="cs", bufs=6) as pcs,
            tc.tile_pool(name="scr", bufs=2) as pscr,
            tc.tile_pool(name="est", bufs=48) as pest,
            tc.tile_pool(name="osb", bufs=3) as posb,
            tc.tile_pool(name="ps", bufs=2, space="PSUM") as pps,
            tc.tile_pool(name="qk", bufs=2, space="PSUM") as pqk,
            tc.tile_pool(name="pj", bufs=2, space="PSUM") as ppj,
        ):
            # ---------------- constants ----------------
            wq_pk = pconst.tile([C2, D], BF16, tag="wq", name="wq_pk")
            wk_pk = pconst.tile([C2, D], BF16, tag="wk", name="wk_pk")
            wv_pk = pconst.tile([C2, D], BF16, tag="wv", name="wv_pk")
            wq_sb = [wq_pk[:, d * 128:(d + 1) * 128] for d in range(ND)]
            wk_sb = [wk_pk[:, d * 128:(d + 1) * 128] for d in range(ND)]
            wv_sb = [wv_pk[:, d * 128:(d + 1) * 128] for d in range(ND)]
            perm_sb = pconst.tile([C2, C2], BF16, tag="perm", name="perm_sb")
            identB = pconst.tile([128, 128], BF16, tag="identB", name="identB")
            make_identity(nc, identB[:])
            wp_sb = pconst.tile([C2, D], BF16, tag="wp", name="wp_sb")

            # persistent rope'd q/k [channel, T] and v tiles [1|v0 v1|1]
            qr = pconst.tile([C2, T], BF16, tag="qr", name="qr")
            kr = pconst.tile([C2, T], BF16, tag="kr", name="kr")
            v_t = []
            for j in range(NT):
                vt = pconst.tile([128, 2 * C + 2], BF16, tag=f"v{j}",
                                 name=f"v{j}")
                nc.vector.memset(vt[:, 0:1], 1.0)
                nc.vector.memset(vt[:, 2 * C + 1:2 * C + 2], 1.0)
                v_t.append(vt)

            # est bookkeeping: est_ref[b][h][j] = (tile, base) where the
            # column for query q (global) of key-tile j is base + q.
            est_ref = [[{} for _ in range(2)] for _ in range(NI)]
            xt_cur = {}
            o_cur = {}
            b_state = {}

            # ---------------- emission units ----------------
            def unit_Bpre(b):
                # loads two blocks' worth (b, b+1) per DMA; odd b is a no-op
                if b in xt_cur:
                    return
                W2 = 2 * TQB
                cosc = pcs.tile([C2, W2], BF16, tag="cos", name="cosc")
                sinc = pcs.tile([C2, W2], BF16, tag="sin", name="sinc")
                xts = []
                if b == 0:
                    # cold start: single-width tiles so the first QKV matmul
                    # is not gated on 2x-wide transfers; weights and rope
                    # tables woven in after the first few x tiles.
                    for d in range(ND):
                        xt = px.tile([128, W2], BF16, tag="xt", name="xt")
                        nc.sync.dma_start(
                            xt[:, 0:TQB], xT_d[d * 128:(d + 1) * 128,
                                               0:TQB])
                        xts.append(xt)
                        if d == 1:
                            nc.sync.dma_start(wk_pk[:], wk_d[:])
                            nc.sync.dma_start(wv_pk[:], wv_d[:])
                            nc.sync.dma_start(perm_sb[:], perm_d[:])
                    nc.sync.dma_start(cosc[:], cos_d[:, 0:W2])
                    nc.sync.dma_start(sinc[:], sin_d[:, 0:W2])
                    for d in range(ND):
                        nc.sync.dma_start(
                            xts[d][:, TQB:W2],
                            xT_d[d * 128:(d + 1) * 128, TQB:W2])
                else:
                    nc.sync.dma_start(cosc[:], cos_d[:, b * TQB:b * TQB + W2])
                    nc.sync.dma_start(sinc[:], sin_d[:, b * TQB:b * TQB + W2])
                    for d in range(ND):
                        xt = px.tile([128, W2], BF16, tag="xt", name="xt")
                        nc.sync.dma_start(
                            xt[:], xT_d[d * 128:(d + 1) * 128,
                                        b * TQB:b * TQB + W2])
                        xts.append(xt)
                xt_cur[b] = ([t[:, 0:TQB] for t in xts],
                             cosc[:, 0:TQB], sinc[:, 0:TQB])
                xt_cur[b + 1] = ([t[:, TQB:W2] for t in xts],
                                 cosc[:, TQB:W2], sinc[:, TQB:W2])

            def unit_Bmm(b, which):
                xts, cosc, sinc = xt_cur[b]
                w_sb = wq_sb if which == "q" else wk_sb
                pacc = pqk.tile([C2, TQB], F32, tag="qkvp", name="pacc")
                for d in range(ND):
                    nc.tensor.matmul(pacc[:], w_sb[d][:], xts[d][:],
                                     start=(d == 0), stop=(d == ND - 1))
                raw = pscr.tile([C2, TQB], BF16, tag="raw", name="raw")
                nc.vector.tensor_copy(raw[:], pacc[:])
                qc = pscr.tile([C2, TQB], BF16, tag="qc", name="qc")
                nc.vector.tensor_mul(qc[:], raw[:], cosc[:])
                b_state[(b, which)] = (raw, qc, sinc)

            def unit_Brope(b, which):
                raw, qc, sinc = b_state.pop((b, which))
                dst = qr if which == "q" else kr
                prot = pqk.tile([C2, TQB], F32, tag="qkvp", name="prot")
                nc.tensor.matmul(prot[:], perm_sb[:], raw[:],
                                 start=True, stop=True)
                qs = pscr.tile([C2, TQB], BF16, tag="qs", name="qs")
                nc.vector.tensor_mul(qs[:], prot[:], sinc[:])
                nc.vector.tensor_add(dst[:, b * TQB:(b + 1) * TQB],
                                     qc[:], qs[:])

            def unit_Bvmm(b):
                xts, _, _ = xt_cur[b]
                pacc = pqk.tile([C2, TQB], F32, tag="qkvp", name="pvacc")
                for d in range(ND):
                    nc.tensor.matmul(pacc[:], wv_sb[d][:], xts[d][:],
                                     start=(d == 0), stop=(d == ND - 1))
                vraw = pscr.tile([C2, TQB], BF16, tag="vraw", name="vraw")
                nc.vector.tensor_copy(vraw[:], pacc[:])
                b_state[(b, "v")] = vraw
                del xt_cur[b]

            def unit_Bvtr(b):
                vraw = b_state.pop((b, "v"))
                pvt = pqk.tile([C2, TQB], BF16, tag="qkvp", name="pvt")
                for s in range(TQB // 128):
                    nc.tensor.transpose(pvt[:, s * 128:(s + 1) * 128],
                                        vraw[:, s * 128:(s + 1) * 128],
                                        identB[:])
                    j = b * (TQB // 128) + s
                    nc.vector.tensor_copy(v_t[j][:, 1:2 * C + 1],
                                          pvt[:, s * 128:(s + 1) * 128])

            # gpsimd fast-exp: bf16 bits = trunc(s*(128*log2e*scale) + magic)
            FEXP_A = 128.0 * 1.4426950408889634 * scale
            FEXP_B = 128.0 * 127.0 - 7.41 + 0.5

            def unit_S(i, g, h, pool=False):
                j0, j1 = 2 * g, 2 * g + 1
                js = [j for j in (j0, j1) if j < 4 * (i + 1)]
                los = [max(TQB * i, 128 * j) for j in js]
                ws = [TQB * (i + 1) - lo for lo in los]
                offs = list(np.cumsum([0] + ws[:-1]))
                wtot = int(sum(ws))
                pst = pps.tile([128, 2 * TQB], F32, tag="pst", name="pst")
                for j, lo, w, o in zip(js, los, ws, offs):
                    nc.tensor.matmul(
                        pst[:, o:o + w],
                        kr[C * h:C * (h + 1), 128 * j:128 * (j + 1)],
                        qr[C * h:C * (h + 1), lo:lo + w],
                        start=True, stop=True, skip_group_check=True)
                if pool:
                    # offload exp to the idle gpsimd engine (off-diag only):
                    # DVE evicts scores to SBUF, gpsimd applies the bf16
                    # exp bit-hack via int16 tensor_scalar.
                    sco = pscr.tile([128, 2 * TQB], BF16, tag="sco",
                                    name="sco")
                    nc.vector.tensor_copy(sco[:, 0:wtot], pst[:, 0:wtot])
                    est_i = pest.tile([128, 2 * TQB], mybir.dt.int16,
                                      tag="est", name="est")
                    nc.gpsimd.tensor_scalar(
                        est_i[:, 0:wtot], sco[:, 0:wtot],
                        FEXP_A, FEXP_B,
                        mybir.AluOpType.mult, mybir.AluOpType.add)
                    est_t = est_i.bitcast(BF16)
                else:
                    est_t = pest.tile([128, 2 * TQB], BF16, tag="est",
                                      name="est")
                    nc.scalar.activation(est_t[:, 0:wtot], pst[:, 0:wtot],
                                         mybir.ActivationFunctionType.Exp,
                                         scale=scale)
                for j, lo, w, o in zip(js, los, ws, offs):
                    if 128 * j >= TQB * i:  # diagonal tile: mask upper tri
                        nc.gpsimd.affine_select(
                            out=est_t[:, o:o + 128],
                            in_=est_t[:, o:o + 128],
                            compare_op=mybir.AluOpType.is_ge,
                            fill=0.0, base=0,
                            pattern=[[1, 128]],
                            channel_multiplier=-1)
                    est_ref[i][h][j] = (est_t, o - lo)

            def unit_Cav(b, qt):
                t = 4 * b + qt
                q0 = 128 * t
                # last two blocks run after all B-phase work: that pool is
                # idle, use it for po so AV chains pipeline independently of
                # the proj evicts.
                po_pool = pqk if b >= NI - 4 else pps
                po_tag = "qkvp" if b >= NI - 4 else "pst"
                po = po_pool.tile([128, 130], F32, tag=po_tag, name="po")
                for h in range(2):
                    for j in range(t + 1):
                        est_t, base = est_ref[b][h][j]
                        col = base + q0
                        if h == 0:
                            outp, rhs = po[:, 0:C + 1], v_t[j][:, 0:C + 1]
                        else:
                            outp, rhs = po[:, C + 1:2 * C + 2], \
                                v_t[j][:, C + 1:2 * C + 2]
                        nc.tensor.matmul(outp, est_t[:, col:col + 128], rhs,
                                         start=(j == 0), stop=(j == t),
                                         skip_group_check=True)
                rcp = pscr.tile([128, 2], F32, tag="rcp", name="rcp")
                nc.vector.reciprocal(rcp[:, 0:1], po[:, 0:1])
                nc.vector.reciprocal(rcp[:, 1:2], po[:, 2 * C + 1:2 * C + 2])
                o_sb = pscr.tile([128, 128], BF16, tag="o_sb", name="o_sb",
                                 bufs=4)
                nc.vector.tensor_scalar_mul(o_sb[:, 0:C], po[:, 1:C + 1],
                                            rcp[:, 0:1])
                nc.vector.tensor_scalar_mul(o_sb[:, C:2 * C],
                                            po[:, C + 1:2 * C + 1],
                                            rcp[:, 1:2])
                o_cur[(b, qt)] = o_sb

            def unit_Cpj(b, qt):
                t = 4 * b + qt
                o_sb = o_cur.pop((b, qt))
                tail = b >= NI - 1
                otp = ppj.tile([128, 128], BF16, tag="pp", name="otp")
                nc.tensor.transpose(otp[:], o_sb[:], identB[:])
                otT = pscr.tile([128, 128], BF16, tag="otT", name="otT")
                nc.vector.tensor_copy(otT[:], otp[:])
                osb = posb.tile([128, D], BF16, tag="osb", name="osb")
                for gc in range(2):
                    pp = ppj.tile([128, TQB], F32, tag="pp", name="ppj")
                    nc.tensor.matmul(pp[:], otT[:],
                                     wp_sb[:, gc * TQB:(gc + 1) * TQB],
                                     start=True, stop=True)
                    nc.vector.tensor_copy(
                        osb[:, gc * TQB:(gc + 1) * TQB], pp[:])
                    if tail:
                        # fire each half as soon as its evict lands so the
                        # drain is not gated on the full row
                        nc.sync.dma_start(
                            out_d[t * 128:(t + 1) * 128,
                                  gc * TQB:(gc + 1) * TQB],
                            osb[:, gc * TQB:(gc + 1) * TQB])
                if not tail:
                    nc.sync.dma_start(out_d[t * 128:(t + 1) * 128, :],
                                      osb[:])

            # ---------------- weave ----------------
            # prologue: x/cos/sin DMAs first so the first QKV matmul can
            # start early, weights packed one DMA per tensor.
            nc.sync.dma_start(wq_pk[:], wq_d[:])
            unit_Bpre(0)
            unit_Bmm(0, "q")
            unit_Bmm(0, "k")
            unit_Brope(0, "q")
            unit_Bvmm(0)
            unit_Brope(0, "k")
            unit_Bvtr(0)
            nc.sync.dma_start(wp_sb[:], wp_d[:])
            # PULL[i] = pairs of block i's scores emitted one iteration early
            # (they only need qr(i), produced mid-way through iteration i-1).
            PULL = [0] + [min(2 * i, 3) for i in range(1, NI)]

            def s_unit(i, g, h):
                # every 3rd off-diagonal pair of late blocks uses the
                # gpsimd fast-exp to offload the Scalar engine
                # gpsimd exp offload measured slower (DVE becomes critical
                # path for the PSUM score eviction); keep everything on ACT.
                pool = False
                return ("S", (i, g, h, pool))

            for i in range(NI):
                npair = 2 * i + 2
                slist = [s_unit(i, g, h)
                         for g in range(PULL[i], npair) for h in range(2)]
                # others, in the order they get woven between S units.
                # Cross-engine chains (Cav->Cpj, Bmm->Brope) are split and
                # separated so the PE never waits on a fresh DVE result.
                b = i + 1
                others = []
                if i + 2 < NI:
                    others += [("Bpre", (i + 2,))]
                if i > 0:
                    others += [("Cav", (i - 1, 0))]
                    others += [("Cav", (i - 1, 1)), ("Cpj", (i - 1, 0))]
                if b < NI:
                    others += [("Bmm", (b, "q"))]
                if i > 0:
                    others += [("Cav", (i - 1, 2)), ("Cpj", (i - 1, 1))]
                if b < NI:
                    others += [("Brope", (b, "q")), ("Bmm", (b, "k"))]
                if i > 0:
                    others += [("Cav", (i - 1, 3)), ("Cpj", (i - 1, 2))]
                if b < NI:
                    others += [("Brope", (b, "k"))]
                    # pulled next-block score pairs: need qr(b) from Brope-q
                    others += [s_unit(b, g, h)
                               for g in range(PULL[b]) for h in range(2)]
                    others += [("Bvmm", (b,))]
                if i > 0:
                    others += [("Cpj", (i - 1, 3))]
                if b < NI:
                    others += [("Bvtr", (b,))]

                ns, no = len(slist), len(others)
                oi = si = 0
                seq = []
                while si < ns or oi < no:
                    if oi < no and (si * no > oi * ns or si >= ns):
                        seq.append(others[oi]); oi += 1
                    else:
                        seq.append(slist[si]); si += 1
                if i == NI - 1:
                    # final block: start its attention as soon as the needed
                    # est tiles exist so the tail overlaps the last scores.
                    out_seq = []
                    sdone = 0
                    for kind, args in seq:
                        if kind == "S":
                            sdone += 1
                        out_seq.append((kind, args))
                        if sdone == ns - 2:
                            out_seq.append(("Cav", (i, 0)))
                            out_seq.append(("Cpj", (i, 0)))
                            out_seq.append(("Cav", (i, 1)))
                            out_seq.append(("Cpj", (i, 1)))
                    for qt in (2, 3):
                        out_seq.append(("Cav", (i, qt)))
                        out_seq.append(("Cpj", (i, qt)))
                    seq = out_seq
                emitters = {"S": unit_S, "Cav": unit_Cav, "Cpj": unit_Cpj,
                            "Bpre": unit_Bpre, "Bmm": unit_Bmm,
                            "Brope": unit_Brope, "Bvmm": unit_Bvmm,
                            "Bvtr": unit_Bvtr}
                for kind, args in seq:
                    emitters[kind](*args)

    nc.compile()
    return nc


def host_inputs(x, W_qkv, W_proj, n_cores=N_CORES):
    """Shard full inputs into per-core input maps (all bf16)."""
    x = np.asarray(x, np.float32)
    W_qkv = np.asarray(W_qkv, np.float32)
    W_proj = np.asarray(W_proj, np.float32)
    T, D = x.shape
    C = C_HEAD
    H = D // C
    HPC = H // n_cores
    C2 = HPC * C
    BF = ml_dtypes.bfloat16
    Wq, Wk, Wv = W_qkv[0:D], W_qkv[D:2 * D], W_qkv[2 * D:3 * D]

    xT = np.ascontiguousarray(x.T.astype(BF))

    # rope tables [C2, T]
    inv_freq = 1.0 / (10000.0 ** (np.arange(0, C, 2, dtype=np.float64) / C))
    ang = np.arange(T, dtype=np.float64)[None, :] * \
        np.repeat(inv_freq, 2)[:, None]          # [C, T]
    cosT = np.ascontiguousarray(np.tile(np.cos(ang), (HPC, 1)).astype(BF))
    sinT = np.ascontiguousarray(np.tile(np.sin(ang), (HPC, 1)).astype(BF))

    # pair-swap-negate permutation: rot = perm.T @ q (within each head block)
    perm = np.zeros((C2, C2), np.float32)
    for cp in range(C2):
        if cp % 2 == 0:
            perm[cp + 1, cp] = -1.0
        else:
            perm[cp - 1, cp] = 1.0
    perm = perm.astype(BF)

    def pack(WT):
        # [D, C2] -> [128, D]: column block d holds rows 128d..128d+127
        return np.ascontiguousarray(
            np.concatenate([WT[128 * d:128 * (d + 1)] for d in range(D // 128)],
                           axis=1).astype(BF))

    in_maps = []
    for c in range(n_cores):
        rows = slice(c * C2, (c + 1) * C2)
        in_maps.append({
            "xT": xT,
            "wq": pack(Wq[rows].T),
            "wk": pack(Wk[rows].T),
            "wv": pack(Wv[rows].T),
            "perm": perm,
            "cosT": cosT,
            "sinT": sinT,
            "wp": np.ascontiguousarray(W_proj[:, rows].T.astype(BF)),
        })
    return in_maps


_PROGRAM_CACHE = {}


def _get_program(T, D, use_f32r=True):
    key = (T, D)
    if key not in _PROGRAM_CACHE:
        _PROGRAM_CACHE[key] = build_program(T=T, D=D)
    return _PROGRAM_CACHE[key]


def run_cores(x, W_qkv, W_proj, **run_kwargs):
    """Run the SPMD program on 8 cores, return BassKernelResults."""
    nc = _get_program(x.shape[0], x.shape[1])
    in_maps = host_inputs(x, W_qkv, W_proj)
    return run_bass_kernel_spmd(nc, in_maps, core_ids=list(range(N_CORES)),
                                **run_kwargs)


def kernel(x, W_qkv, W_proj):
    res = run_cores(x, W_qkv, W_proj)
    out = np.zeros((x.shape[0], x.shape[1]), np.float32)
    for r in res.results:
        out += np.asarray(r["out"], dtype=np.float32)
    return out
